# revision 20
# baseline (speedup 1.0000x reference)
"""Multi-head self-attention (B=2, N=2048, C=1024, H=16) on 8 TRN2 NeuronCores.

Sharding: data-parallel over batch (2) x tensor-parallel over heads (16/4=4 groups).
Core c handles batch b=c//4 and heads [4*(c%4), 4*(c%4)+4).

v5 schedule. The scalar engine (softmax exp: 128 ACTIVATEs x ~1.1us = 142us) is
the critical resource; PE streaming (~137us) hides under it. DMA descriptor
issue costs ~0.7us per dma_start on the issuing engine, so all inputs load via
EIGHT long-line 2D DMAs (SBUF layouts are chosen so each transfer is contiguous
per partition):
  - x^T as [128p][4 sbk][8 ct][512]: one 1MB DMA per seq-superblock (8KB lines)
    on the sync queue, in consumption order -> compute starts after ~1.5MB.
  - w_qkv as [128p][3 blk][8 ct][256] with blocks (k01|q01, v, k23|q23): three
    0.5MB DMAs on the scalar queue, first-needed first. w_out similarly.
Priority bands steer the Tile list scheduler: band 0 = exp-table preload dummy
+ input DMAs + QKV projection groups; high_priority band = the attention
S->exp->PV->normalize chain so a ready S matmul always beats background work;
out-projection per seq-range is emitted at natural priority and fills PE slack.
Attention runs in a skewed (ic, pair) order so pair-1 prerequisites (k23/q23)
are produced in earlier windows' slack; the final out-projection chunk is the
only true tail: its PSUM->SBUF copies alternate vector/scalar engines (ACT is
idle after the last exp) and its PSUM rotation reuses the freed po banks.

Per-core math identical to v1: S^T pairs row-packed into disjoint 64-row PE
groups, P^T = exp(S/8) on ACT, O_aug^T = [V|1]^T P^T accumulated over key tiles
(ones column yields softmax sums), normalize via reciprocal+partition_broadcast,
Y = O_norm @ W_out. Host sums the 4 head-group partials per batch (+b_out).
"""

import contextlib

import numpy as np

import concourse.bass as bass
import concourse.bacc as bacc
import concourse.tile as tile
from concourse import library_config, mybir
from concourse.bass_utils import run_bass_kernel_spmd

B, NSEQ, CDIM, NHEADS, HD = 2, 2048, 1024, 16, 64
NH = 4          # heads per core
NCORES = 8
F32 = mybir.dt.float32
F16 = mybir.dt.float16  # 16-bit matmul dtype (10-bit mantissa, ample range here)
EXP = mybir.ActivationFunctionType.Exp
SCALE = HD ** -0.5
PRIO_OFF = 1_000_000
FEXP_A = 1024.0 * SCALE / 0.6931471805599453  # log2(e)*2^10*scale
FEXP_B = 15.0 * 1024.0 - 44.25                # f16 exp bias - spline shift


def build_program():
    nc = bacc.Bacc("TRN2", target_bir_lowering=False, debug=False)

    xt4 = nc.dram_tensor("xt4", [4, 128, 8, 512], F16, kind="ExternalInput").ap()
    wkq01 = nc.dram_tensor("wkq01", [128, 8, 256], F16, kind="ExternalInput").ap()
    wv4 = nc.dram_tensor("wv4", [128, 8, 256], F16, kind="ExternalInput").ap()
    wkq23 = nc.dram_tensor("wkq23", [128, 8, 256], F16, kind="ExternalInput").ap()
    wout2 = nc.dram_tensor("wout2", [128, 2, CDIM], F16,
                           kind="ExternalInput").ap()
    y = nc.dram_tensor("y", [NSEQ, CDIM], F16, kind="ExternalOutput").ap()

    with tile.TileContext(nc) as tc:
        emit(nc, tc, xt4, wkq01, wv4, wkq23, wout2, y)

    nc.compile()
    return nc


def emit(nc, tc, xt4, wkq01, wv4, wkq23, wout2, y):
    ctx = contextlib.ExitStack()
    with ctx:
        const = ctx.enter_context(tc.tile_pool(name="const", bufs=1))

        # ---- persistent SBUF tensors (DMA lands here directly, f16) ----
        # wqkv blocks: 0 = [k01|q01], 1 = v (4 heads), 2 = [k23|q23]
        wqkv_sb = const.tile([128, 3, 8, 256], F16)         # [p, blk, ct, 256]
        wout_sb = const.tile([128, 2, CDIM], F16)           # [p, kt, 1024]
        xT_sb = const.tile([128, 4, 8, 512], F16)           # [p, sbk, ct, 512]
        qk_sb = const.tile([128, 4, NSEQ], F16)             # dim1: q01,q23,k01,k23
        v_aug = const.tile([128, 16, NH, HD + 1], F16)      # [p, ntile, head, V|1]
        o_sb = const.tile([128, 2, NSEQ], F16)              # normalized O^T, pairs
        dum = const.tile([1, 8], F32)
        dumw = const.tile([128, 512], F16)                  # PE warm-up operand
        one16 = const.tile([1, 64], F16)                    # PE-broadcast ones

        nc.gpsimd.load_library(library_config.attn)
        nc.vector.memset(v_aug[:, :, :, HD:HD + 1], 1.0)
        nc.vector.memset(dum, 1.0)
        nc.vector.memset(dumw, 0.0)
        nc.vector.memset(one16, 1.0)

        with tc.tile_pool(name="pP", bufs=4) as pP, \
             tc.tile_pool(name="oup", bufs=2) as oup, \
             tc.tile_pool(name="stat", bufs=2) as stat, \
             tc.tile_pool(name="rbc", bufs=4) as rbc, \
             tc.tile_pool(name="shf", bufs=2) as shf, \
             tc.tile_pool(name="yb", bufs=3) as yb, \
             tc.tile_pool(name="psm", bufs=1, space="PSUM") as psm:

            # exp table preload: a dummy ACTIVATE with no data deps runs first
            # on the scalar engine so the ~2.7us ACT_TABLE_LOAD is off the
            # critical path.
            dume = stat.tile([1, 8], F16, tag="dume", name="dume")
            nc.scalar.activation(dume, dum, EXP)

            # ---- input DMAs: long-line transfers, both HWDGE queues, in
            # consumption order. The first seq-superblock goes per-ct so the
            # first QKV groups chase the transfers instead of waiting on one
            # 1MB descriptor.
            for ct in range(8):
                nc.sync.dma_start(xT_sb[:, 0, ct, :], xt4[0, :, ct, :])
            for sbk in range(1, 4):
                nc.sync.dma_start(xT_sb[:, sbk], xt4[sbk])
            nc.scalar.dma_start(wqkv_sb[:, 0], wkq01)
            nc.scalar.dma_start(wqkv_sb[:, 1], wv4)
            nc.scalar.dma_start(wqkv_sb[:, 2], wkq23)
            nc.scalar.dma_start(wout_sb, wout2)

            # PE warm-up: dummy matmuls with no data deps keep the PE busy
            # during the input DMA window so HAM reaches 2.4GHz before real
            # work, and the first real matmuls issue back-to-back warm.
            wrm = psm.tile([128, 512], F32, tag="vp", bufs=1, name="wrm")
            for r in range(12):
                nc.tensor.matmul(wrm, dumw[:, 0:128], dumw,
                                 start=(r == 0), stop=(r == 11))

            TB = {"qk": 1, "vp": 1, "sb": 2, "o0": 1, "o1": 1}

            def qk_group(blk, half, ft, ic, tag):
                ps = psm.tile([128, 512], F32, tag=tag, bufs=TB[tag], name="psqk")
                for ct in range(8):
                    nc.tensor.matmul(
                        ps,
                        wqkv_sb[:, blk, ct, half * 128:half * 128 + 128],
                        xT_sb[:, ic, ct, :],
                        start=(ct == 0), stop=(ct == 7),
                    )
                nc.vector.tensor_copy(qk_sb[:, ft, ic * 512:(ic + 1) * 512], ps)

            def v_group(nt, tag):
                ps = psm.tile([128, 512], F32, tag=tag, bufs=TB[tag], name="psvp")
                for ct in range(8):
                    nc.tensor.matmul(
                        ps[:, 0:NH * HD],
                        xT_sb[:, nt // 4, ct,
                              (nt % 4) * 128:(nt % 4) * 128 + 128],
                        wqkv_sb[:, 1, ct, :],
                        start=(ct == 0), stop=(ct == 7),
                    )
                for h in range(NH):
                    nc.vector.tensor_copy(
                        v_aug[:, nt, h, 0:HD], ps[:, h * HD:(h + 1) * HD]
                    )

            def y_tile(it, tags, engs):
                """Both 512-wide output chunks for one seq tile, one store."""
                y_sb = yb.tile([128, CDIM], F16, tag="ysb", name="ysbt")
                for fc in range(2):
                    psy = psm.tile([128, 512], F32, tag=tags[fc],
                                   bufs=TB[tags[fc]], name="pyt")
                    for pp in range(2):
                        nc.tensor.matmul(
                            psy,
                            o_sb[:, pp, it * 128:(it + 1) * 128],
                            wout_sb[:, pp, fc * 512:(fc + 1) * 512],
                            start=(pp == 0), stop=(pp == 1),
                        )
                    if engs[fc] == "scalar":
                        nc.scalar.copy(y_sb[:, fc * 512:(fc + 1) * 512], psy)
                    else:
                        nc.vector.tensor_copy(
                            y_sb[:, fc * 512:(fc + 1) * 512], psy)
                nc.sync.dma_start(y[it * 128:(it + 1) * 128, :], y_sb)

            # ---- band 0: QKV projection, emitted in the order the skewed
            # attention windows consume it: k01/q01/V for pair-0 windows
            # first, then q01(ic>=1), then k23/q23 for the delayed pair-1
            # windows. Priorities (= emission order) make background PE work
            # track attention consumption.
            tg = ["qk", "vp"]
            t = 0

            def nxt():
                nonlocal t
                t += 1
                return tg[t % 2]

            qk_group(0, 0, 2, 0, nxt())          # k01(0)
            qk_group(0, 1, 0, 0, nxt())          # q01(0)
            for nt in range(0, 4):
                v_group(nt, nxt())
            for sbk in range(1, 4):
                qk_group(0, 0, 2, sbk, nxt())    # k01(sbk)
                for nt in range(4 * sbk, 4 * sbk + 4):
                    v_group(nt, nxt())

            # Late-needed projection groups + finished seq-ranges' y tiles are
            # sprinkled into earlier windows' jt-chunk boundaries (emission
            # position = scheduler priority): background production tracks
            # attention consumption without head-of-line blocking the
            # in-order PE stream. Window order index -> list of closures.
            sprk = {
                0: [lambda: qk_group(0, 1, 0, 1, nxt()),    # q01(1) -> w1
                    lambda: qk_group(2, 0, 3, 0, nxt()),    # k23(0) -> w2
                    lambda: qk_group(2, 0, 3, 1, nxt()),
                    lambda: qk_group(2, 0, 3, 2, nxt())],
                1: [lambda: qk_group(2, 0, 3, 3, nxt()),
                    lambda: qk_group(2, 1, 1, 0, nxt()),    # q23(0) -> w2
                    lambda: qk_group(0, 1, 0, 2, nxt()),    # q01(2) -> w3
                    lambda: qk_group(2, 1, 1, 1, nxt())],   # q23(1) -> w4
                2: [lambda: qk_group(0, 1, 0, 3, nxt()),    # q01(3) -> w5
                    lambda: qk_group(2, 1, 1, 2, nxt()),    # q23(2) -> w6
                    lambda: qk_group(2, 1, 1, 3, nxt())],   # q23(3) -> w7
            }

            # ---- attention: skewed (ic, pair) order — delays each pair-1
            # pass one window so k23/q23/V production absorbs into slack.
            # y(ic) tiles are sprinkled into the NEXT window's jt loop (their
            # deps are a full window old by then — no head-of-line blocking
            # of the in-order PE stream).
            order = [(0, 0), (1, 0), (0, 1), (2, 0), (1, 1), (3, 0),
                     (2, 1), (3, 1)]
            for widx, (ic, p) in enumerate(order):
                pend = sprk.get(widx, [])
                i0 = ic * 512
                last = widx == len(order) - 1
                po = None
                for jc in range(4):  # jt chunks of 4
                    with tc.high_priority(offset=PRIO_OFF):
                        if po is None:
                            po = [psm.tile([128, 512], F32, tag=f"o{e}",
                                           name=f"po{e}")[0:HD + 1, :]
                                  for e in range(2)]
                        for jt in range(4 * jc, 4 * jc + 4):  # key tile (128)
                            ps = psm.tile([128, 1024], F32, tag="sb", bufs=2,
                                          name="pss")
                            for e in range(2):  # row-group packed pair
                                pb = 64 * e
                                nc.tensor.matmul(
                                    ps[:, e * 512:(e + 1) * 512],
                                    qk_sb[pb:pb + 64, 2 + p,
                                          jt * 128:(jt + 1) * 128],
                                    qk_sb[pb:pb + 64, p, i0:i0 + 512],
                                    start=True, stop=True,
                                    tile_position=(pb, 0),
                                )
                            pt = pP.tile([128, 1024], F16, tag="p")
                            nc.scalar.activation(pt, ps, EXP, scale=SCALE)
                            for e in range(2):
                                nc.tensor.matmul(
                                    po[e],
                                    v_aug[:, jt, 2 * p + e, :],
                                    pt[:, e * 512:(e + 1) * 512],
                                    start=(jt == 0), stop=(jt == 15),
                                )
                    if last and jc == 3:
                        # no-dependency matmuls emitted right behind the
                        # final PV keep the PE warm and busy while the last
                        # normalize chain (DMA hop + reciprocal + broadcast)
                        # drains, so the closing out-projection runs at full
                        # clock instead of re-throttled.
                        with tc.high_priority(offset=PRIO_OFF):
                            wr2 = psm.tile([128, 1024], F32, tag="sb",
                                           bufs=2, name="wr2")
                            for r in range(28):
                                nc.tensor.matmul(
                                    wr2[:, 0:512], dumw[:, 0:128], dumw,
                                    start=(r == 0), stop=(r == 27))
                    if pend:
                        fn = pend.pop(0)
                        if fn is not None:
                            fn()
                with tc.high_priority(offset=PRIO_OFF):
                    # normalize: copy out of PSUM immediately (frees po
                    # banks), then ONE reciprocal + ONE partition_broadcast
                    # over both heads' concatenated sums rows, then multiply.
                    # e=1 first so its o_sb shift DMA overlaps e=0's mul; the
                    # last window's copies split across scalar+vector (ACT is
                    # idle after the final exp).
                    o_us = [None, None]
                    r0 = stat.tile([1, 1024], F32, tag="r0", name="r0")
                    for e in (1, 0):
                        o_u = oup.tile([HD + 1, 512], F32, tag=f"ou{e}",
                                       name=f"ou{e}")
                        o_us[e] = o_u
                        if last and e == 1:
                            nc.scalar.copy(o_u, po[e])
                        else:
                            nc.vector.tensor_copy(o_u, po[e])
                        (nc.scalar if last else nc.sync).dma_start(
                            r0[:, e * 512:(e + 1) * 512], o_u[HD:HD + 1, :])
                    r1 = stat.tile([1, 1024], F32, tag="r1", name="r1")
                    nc.vector.reciprocal_approx_fast(r1, r0)
                    rb = rbc.tile([64, 1024], F32, tag="rb")
                    nc.gpsimd.partition_broadcast(rb, r1)
                    tmp = shf.tile([64, 512], F16, tag="tmp")
                    nc.vector.tensor_mul(tmp, o_us[1][0:64, :],
                                         rb[:, 512:1024])
                    nc.sync.dma_start(o_sb[64:128, p, i0:i0 + 512], tmp)
                    nc.vector.tensor_mul(o_sb[0:64, p, i0:i0 + 512],
                                         o_us[0][0:64, :], rb[:, 0:512])
                for fn in pend:  # leftovers
                    if fn is not None:
                        fn()
                if p == 1:
                    tiles = []
                    for j, it in enumerate(range(4 * ic, 4 * ic + 4)):
                        engs = ("scalar", "vector") if ic >= 2 else (
                            "vector", "vector")
                        if ic == 3 and j % 2 == 1:
                            # only the very last seq range may rotate through
                            # the po banks — they are busy until the final
                            # normalize for any earlier range
                            tags = ("o0", "o1")
                        else:
                            tags = ("qk", "vp")
                        tiles.append(
                            lambda it=it, tags=tags, engs=engs:
                                y_tile(it, tags, engs))
                    # first two into the next window's LATE chunk slots (the
                    # normalize chain incl. the o_sb shift DMA must be done
                    # for real before the in-order PE stream meets them);
                    # last two a full window later.
                    sprk.setdefault(widx + 1, []).extend(
                        [None, None, tiles[0], tiles[1]])
                    sprk.setdefault(widx + 2, []).extend(tiles[2:])
            # the final seq ranges' out-projection is the only true tail
            for k in (len(order), len(order) + 1):
                for fn in sprk.get(k, []):
                    if fn is not None:
                        fn()



_NC = None


def _get_nc():
    global _NC
    if _NC is None:
        _NC = build_program()
    return _NC


def make_in_maps(x, w_qkv, w_out):
    x = np.asarray(x, dtype=np.float32)
    w_qkv = np.asarray(w_qkv, dtype=np.float32)
    w_out = np.asarray(w_out, dtype=np.float32)
    xt4 = []
    for b in range(B):
        xT = x[b].T.astype(np.float16)  # [C, N] = [(ct p), (sbk s)]
        xt4.append(np.ascontiguousarray(
            xT.reshape(8, 128, 4, 512).transpose(2, 1, 0, 3)))
    in_maps = []
    for c in range(NCORES):
        b, g = divmod(c, 4)
        f0 = g * NH * HD  # first feature col of this head group (256 wide)
        wq = w_qkv[:, f0:f0 + 256]
        wk = w_qkv[:, CDIM + f0:CDIM + f0 + 256]
        wv = w_qkv[:, 2 * CDIM + f0:2 * CDIM + f0 + 256]

        def blk(a):  # [1024, 256] f32 -> [128 p, 8 ct, 256] f16 contiguous
            return np.ascontiguousarray(
                a.astype(np.float16).reshape(8, 128, 256).transpose(1, 0, 2))

        in_maps.append({
            "xt4": xt4[b],
            "wkq01": blk(np.concatenate([wk[:, :128], wq[:, :128]], axis=1)),
            "wv4": blk(wv),
            "wkq23": blk(np.concatenate([wk[:, 128:], wq[:, 128:]], axis=1)),
            "wout2": np.ascontiguousarray(
                w_out[f0:f0 + 256, :].astype(np.float16)
                .reshape(2, 128, CDIM).transpose(1, 0, 2)),
        })
    return in_maps


def kernel(x, w_qkv, b_qkv, w_out, b_out, _trace=False):
    """Full inputs in, full (B, N, C) output out. b_qkv is all-zeros by the
    problem's input spec (fill: zeros); b_out is added on the host."""
    nc = _get_nc()
    in_maps = make_in_maps(x, w_qkv, w_out)
    res = run_bass_kernel_spmd(nc, in_maps, core_ids=list(range(NCORES)),
                               trace=_trace)
    out = np.zeros((B, NSEQ, CDIM), dtype=np.float32)
    for c in range(NCORES):
        out[c // 4] += res.results[c]["y"].astype(np.float32)
    out += np.asarray(b_out, dtype=np.float32)
    if _trace:
        kernel.last_exec_time_ns = res.exec_time_ns
        kernel.last_results = res
    return out


# revision 21
# speedup vs baseline: 1.0062x; 1.0062x over previous
"""Multi-head self-attention (B=2, N=2048, C=1024, H=16) on 8 TRN2 NeuronCores.

Sharding: data-parallel over batch (2) x tensor-parallel over heads (16/4=4 groups).
Core c handles batch b=c//4 and heads [4*(c%4), 4*(c%4)+4).

v5 schedule. The scalar engine (softmax exp: 128 ACTIVATEs x ~1.1us = 142us) is
the critical resource; PE streaming (~137us) hides under it. DMA descriptor
issue costs ~0.7us per dma_start on the issuing engine, so all inputs load via
EIGHT long-line 2D DMAs (SBUF layouts are chosen so each transfer is contiguous
per partition):
  - x^T as [128p][4 sbk][8 ct][512]: one 1MB DMA per seq-superblock (8KB lines)
    on the sync queue, in consumption order -> compute starts after ~1.5MB.
  - w_qkv as [128p][3 blk][8 ct][256] with blocks (k01|q01, v, k23|q23): three
    0.5MB DMAs on the scalar queue, first-needed first. w_out similarly.
Priority bands steer the Tile list scheduler: band 0 = exp-table preload dummy
+ input DMAs + QKV projection groups; high_priority band = the attention
S->exp->PV->normalize chain so a ready S matmul always beats background work;
out-projection per seq-range is emitted at natural priority and fills PE slack.
Attention runs in a skewed (ic, pair) order so pair-1 prerequisites (k23/q23)
are produced in earlier windows' slack; the final out-projection chunk is the
only true tail: its PSUM->SBUF copies alternate vector/scalar engines (ACT is
idle after the last exp) and its PSUM rotation reuses the freed po banks.

Per-core math identical to v1: S^T pairs row-packed into disjoint 64-row PE
groups, P^T = exp(S/8) on ACT, O_aug^T = [V|1]^T P^T accumulated over key tiles
(ones column yields softmax sums), normalize via reciprocal+partition_broadcast,
Y = O_norm @ W_out. Host sums the 4 head-group partials per batch (+b_out).
"""

import contextlib

import numpy as np

import concourse.bass as bass
import concourse.bacc as bacc
import concourse.tile as tile
from concourse import library_config, mybir
from concourse.bass_utils import run_bass_kernel_spmd

B, NSEQ, CDIM, NHEADS, HD = 2, 2048, 1024, 16, 64
NH = 4          # heads per core
NCORES = 8
F32 = mybir.dt.float32
F16 = mybir.dt.float16  # 16-bit matmul dtype (10-bit mantissa, ample range here)
EXP = mybir.ActivationFunctionType.Exp
SCALE = HD ** -0.5
PRIO_OFF = 1_000_000
FEXP_A = 1024.0 * SCALE / 0.6931471805599453  # log2(e)*2^10*scale
FEXP_B = 15.0 * 1024.0 - 44.25                # f16 exp bias - spline shift


def build_program():
    nc = bacc.Bacc("TRN2", target_bir_lowering=False, debug=False)

    xt4 = nc.dram_tensor("xt4", [4, 128, 8, 512], F16, kind="ExternalInput").ap()
    wkq01 = nc.dram_tensor("wkq01", [128, 8, 256], F16, kind="ExternalInput").ap()
    wv4 = nc.dram_tensor("wv4", [128, 8, 256], F16, kind="ExternalInput").ap()
    wkq23 = nc.dram_tensor("wkq23", [128, 8, 256], F16, kind="ExternalInput").ap()
    wout2 = nc.dram_tensor("wout2", [128, 2, CDIM], F16,
                           kind="ExternalInput").ap()
    y = nc.dram_tensor("y", [NSEQ, CDIM], F16, kind="ExternalOutput").ap()

    with tile.TileContext(nc) as tc:
        emit(nc, tc, xt4, wkq01, wv4, wkq23, wout2, y)

    nc.compile()
    return nc


def emit(nc, tc, xt4, wkq01, wv4, wkq23, wout2, y):
    ctx = contextlib.ExitStack()
    with ctx:
        const = ctx.enter_context(tc.tile_pool(name="const", bufs=1))

        # ---- persistent SBUF tensors (DMA lands here directly, f16) ----
        # wqkv blocks: 0 = [k01|q01], 1 = v (4 heads), 2 = [k23|q23]
        wqkv_sb = const.tile([128, 3, 8, 256], F16)         # [p, blk, ct, 256]
        wout_sb = const.tile([128, 2, CDIM], F16)           # [p, kt, 1024]
        xT_sb = const.tile([128, 4, 8, 512], F16)           # [p, sbk, ct, 512]
        qk_sb = const.tile([128, 4, NSEQ], F16)             # dim1: q01,q23,k01,k23
        v_aug = const.tile([128, 16, NH, HD + 1], F16)      # [p, ntile, head, V|1]
        o_sb = const.tile([128, 2, NSEQ], F16)              # normalized O^T, pairs
        dum = const.tile([1, 8], F32)
        dumw = const.tile([128, 512], F16)                  # PE warm-up operand

        nc.gpsimd.load_library(library_config.attn)
        nc.vector.memset(v_aug[:, :, :, HD:HD + 1], 1.0)
        nc.vector.memset(dum, 1.0)
        nc.vector.memset(dumw, 0.0)

        with tc.tile_pool(name="pP", bufs=4) as pP, \
             tc.tile_pool(name="oup", bufs=2) as oup, \
             tc.tile_pool(name="stat", bufs=2) as stat, \
             tc.tile_pool(name="rbc", bufs=4) as rbc, \
             tc.tile_pool(name="shf", bufs=2) as shf, \
             tc.tile_pool(name="yb", bufs=3) as yb, \
             tc.tile_pool(name="psm", bufs=1, space="PSUM") as psm:

            # exp table preload: a dummy ACTIVATE with no data deps runs first
            # on the scalar engine so the ~2.7us ACT_TABLE_LOAD is off the
            # critical path.
            dume = stat.tile([1, 8], F16, tag="dume", name="dume")
            nc.scalar.activation(dume, dum, EXP)

            # ---- input DMAs: long-line transfers, both HWDGE queues, in
            # consumption order. The first seq-superblock goes per-ct so the
            # first QKV groups chase the transfers instead of waiting on one
            # 1MB descriptor.
            for ct in range(8):
                nc.sync.dma_start(xT_sb[:, 0, ct, :], xt4[0, :, ct, :])
            for sbk in range(1, 4):
                nc.sync.dma_start(xT_sb[:, sbk], xt4[sbk])
            nc.scalar.dma_start(wqkv_sb[:, 0], wkq01)
            nc.scalar.dma_start(wqkv_sb[:, 1], wv4)
            nc.scalar.dma_start(wqkv_sb[:, 2], wkq23)
            nc.scalar.dma_start(wout_sb, wout2)

            # PE warm-up: dummy matmuls with no data deps keep the PE busy
            # during the input DMA window so HAM reaches 2.4GHz before real
            # work, and the first real matmuls issue back-to-back warm.
            wrm = psm.tile([128, 512], F32, tag="vp", bufs=1, name="wrm")
            for r in range(12):
                nc.tensor.matmul(wrm, dumw[:, 0:128], dumw,
                                 start=(r == 0), stop=(r == 11))

            TB = {"qk": 1, "vp": 1, "sb": 2, "o0": 1, "o1": 1}

            def qk_group(blk, half, ft, ic, tag):
                ps = psm.tile([128, 512], F32, tag=tag, bufs=TB[tag], name="psqk")
                for ct in range(8):
                    nc.tensor.matmul(
                        ps,
                        wqkv_sb[:, blk, ct, half * 128:half * 128 + 128],
                        xT_sb[:, ic, ct, :],
                        start=(ct == 0), stop=(ct == 7),
                    )
                nc.vector.tensor_copy(qk_sb[:, ft, ic * 512:(ic + 1) * 512], ps)

            def v_group(nt, tag):
                ps = psm.tile([128, 512], F32, tag=tag, bufs=TB[tag], name="psvp")
                for ct in range(8):
                    nc.tensor.matmul(
                        ps[:, 0:NH * HD],
                        xT_sb[:, nt // 4, ct,
                              (nt % 4) * 128:(nt % 4) * 128 + 128],
                        wqkv_sb[:, 1, ct, :],
                        start=(ct == 0), stop=(ct == 7),
                    )
                for h in range(NH):
                    nc.vector.tensor_copy(
                        v_aug[:, nt, h, 0:HD], ps[:, h * HD:(h + 1) * HD]
                    )

            def y_tile(it, tags, engs):
                """Both 512-wide output chunks for one seq tile, one store."""
                y_sb = yb.tile([128, CDIM], F16, tag="ysb", name="ysbt")
                for fc in range(2):
                    psy = psm.tile([128, 512], F32, tag=tags[fc],
                                   bufs=TB[tags[fc]], name="pyt")
                    for pp in range(2):
                        nc.tensor.matmul(
                            psy,
                            o_sb[:, pp, it * 128:(it + 1) * 128],
                            wout_sb[:, pp, fc * 512:(fc + 1) * 512],
                            start=(pp == 0), stop=(pp == 1),
                        )
                    if engs[fc] == "scalar":
                        nc.scalar.copy(y_sb[:, fc * 512:(fc + 1) * 512], psy)
                    else:
                        nc.vector.tensor_copy(
                            y_sb[:, fc * 512:(fc + 1) * 512], psy)
                nc.sync.dma_start(y[it * 128:(it + 1) * 128, :], y_sb)

            # ---- band 0: QKV projection, emitted in the order the skewed
            # attention windows consume it: k01/q01/V for pair-0 windows
            # first, then q01(ic>=1), then k23/q23 for the delayed pair-1
            # windows. Priorities (= emission order) make background PE work
            # track attention consumption.
            tg = ["qk", "vp"]
            t = 0

            def nxt():
                nonlocal t
                t += 1
                return tg[t % 2]

            qk_group(0, 0, 2, 0, nxt())          # k01(0)
            qk_group(0, 1, 0, 0, nxt())          # q01(0)
            for nt in range(0, 4):
                v_group(nt, nxt())
            for sbk in range(1, 4):
                qk_group(0, 0, 2, sbk, nxt())    # k01(sbk)
                for nt in range(4 * sbk, 4 * sbk + 4):
                    v_group(nt, nxt())

            # Late-needed projection groups + finished seq-ranges' y tiles are
            # sprinkled into earlier windows' jt-chunk boundaries (emission
            # position = scheduler priority): background production tracks
            # attention consumption without head-of-line blocking the
            # in-order PE stream. Window order index -> list of closures.
            sprk = {
                0: [lambda: qk_group(0, 1, 0, 1, nxt()),    # q01(1) -> w1
                    lambda: qk_group(2, 0, 3, 0, nxt()),    # k23(0) -> w2
                    lambda: qk_group(2, 0, 3, 1, nxt()),
                    lambda: qk_group(2, 0, 3, 2, nxt())],
                1: [lambda: qk_group(2, 0, 3, 3, nxt()),
                    lambda: qk_group(2, 1, 1, 0, nxt()),    # q23(0) -> w2
                    lambda: qk_group(0, 1, 0, 2, nxt()),    # q01(2) -> w3
                    lambda: qk_group(2, 1, 1, 1, nxt())],   # q23(1) -> w4
                2: [lambda: qk_group(0, 1, 0, 3, nxt()),    # q01(3) -> w5
                    lambda: qk_group(2, 1, 1, 2, nxt()),    # q23(2) -> w6
                    lambda: qk_group(2, 1, 1, 3, nxt())],   # q23(3) -> w7
            }

            # ---- attention: skewed (ic, pair) order — delays each pair-1
            # pass one window so k23/q23/V production absorbs into slack.
            # y(ic) tiles are sprinkled into the NEXT window's jt loop (their
            # deps are a full window old by then — no head-of-line blocking
            # of the in-order PE stream).
            order = [(0, 0), (1, 0), (0, 1), (2, 0), (1, 1), (3, 0),
                     (2, 1), (3, 1)]
            for widx, (ic, p) in enumerate(order):
                pend = sprk.get(widx, [])
                i0 = ic * 512
                last = widx == len(order) - 1
                po = None
                for jc in range(4):  # jt chunks of 4
                    with tc.high_priority(offset=PRIO_OFF):
                        if po is None:
                            po = [psm.tile([128, 512], F32, tag=f"o{e}",
                                           name=f"po{e}")[0:HD + 1, :]
                                  for e in range(2)]
                        for jt in range(4 * jc, 4 * jc + 4):  # key tile (128)
                            ps = psm.tile([128, 1024], F32, tag="sb", bufs=2,
                                          name="pss")
                            for e in range(2):  # row-group packed pair
                                pb = 64 * e
                                nc.tensor.matmul(
                                    ps[:, e * 512:(e + 1) * 512],
                                    qk_sb[pb:pb + 64, 2 + p,
                                          jt * 128:(jt + 1) * 128],
                                    qk_sb[pb:pb + 64, p, i0:i0 + 512],
                                    start=True, stop=True,
                                    tile_position=(pb, 0),
                                )
                            pt = pP.tile([128, 1024], F16, tag="p")
                            nc.scalar.activation(pt, ps, EXP, scale=SCALE)
                            for e in range(2):
                                nc.tensor.matmul(
                                    po[e],
                                    v_aug[:, jt, 2 * p + e, :],
                                    pt[:, e * 512:(e + 1) * 512],
                                    start=(jt == 0), stop=(jt == 15),
                                )
                    if pend:
                        fn = pend.pop(0)
                        if fn is not None:
                            fn()
                with tc.high_priority(offset=PRIO_OFF):
                    # normalize: copy out of PSUM immediately (frees po
                    # banks), then ONE reciprocal + ONE partition_broadcast
                    # over both heads' concatenated sums rows, then multiply.
                    # e=1 first so its o_sb shift DMA overlaps e=0's mul; the
                    # last window's copies split across scalar+vector (ACT is
                    # idle after the final exp).
                    o_us = [None, None]
                    r0 = stat.tile([1, 1024], F32, tag="r0", name="r0")
                    for e in (1, 0):
                        o_u = oup.tile([HD + 1, 512], F32, tag=f"ou{e}",
                                       name=f"ou{e}")
                        o_us[e] = o_u
                        if last and e == 1:
                            nc.scalar.copy(o_u, po[e])
                        else:
                            nc.vector.tensor_copy(o_u, po[e])
                        nc.sync.dma_start(r0[:, e * 512:(e + 1) * 512],
                                          o_u[HD:HD + 1, :])
                    r1 = stat.tile([1, 1024], F32, tag="r1", name="r1")
                    nc.vector.reciprocal_approx_fast(r1, r0)
                    rb = rbc.tile([64, 1024], F32, tag="rb")
                    nc.gpsimd.partition_broadcast(rb, r1)
                    tmp = shf.tile([64, 512], F16, tag="tmp")
                    nc.vector.tensor_mul(tmp, o_us[1][0:64, :],
                                         rb[:, 512:1024])
                    nc.sync.dma_start(o_sb[64:128, p, i0:i0 + 512], tmp)
                    nc.vector.tensor_mul(o_sb[0:64, p, i0:i0 + 512],
                                         o_us[0][0:64, :], rb[:, 0:512])
                for fn in pend:  # leftovers
                    if fn is not None:
                        fn()
                if p == 1:
                    tiles = []
                    for j, it in enumerate(range(4 * ic, 4 * ic + 4)):
                        engs = ("scalar", "vector") if ic >= 2 else (
                            "vector", "vector")
                        if ic == 3 and j % 2 == 1:
                            # only the very last seq range may rotate through
                            # the po banks — they are busy until the final
                            # normalize for any earlier range
                            tags = ("o0", "o1")
                        else:
                            tags = ("qk", "vp")
                        tiles.append(
                            lambda it=it, tags=tags, engs=engs:
                                y_tile(it, tags, engs))
                    # first two into the next window's LATE chunk slots (the
                    # normalize chain incl. the o_sb shift DMA must be done
                    # for real before the in-order PE stream meets them);
                    # last two a full window later.
                    sprk.setdefault(widx + 1, []).extend(
                        [None, None, tiles[0], tiles[1]])
                    sprk.setdefault(widx + 2, []).extend(tiles[2:])
            # the final seq ranges' out-projection is the only true tail
            for k in (len(order), len(order) + 1):
                for fn in sprk.get(k, []):
                    if fn is not None:
                        fn()



_NC = None


def _get_nc():
    global _NC
    if _NC is None:
        _NC = build_program()
    return _NC


def make_in_maps(x, w_qkv, w_out):
    x = np.asarray(x, dtype=np.float32)
    w_qkv = np.asarray(w_qkv, dtype=np.float32)
    w_out = np.asarray(w_out, dtype=np.float32)
    xt4 = []
    for b in range(B):
        xT = x[b].T.astype(np.float16)  # [C, N] = [(ct p), (sbk s)]
        xt4.append(np.ascontiguousarray(
            xT.reshape(8, 128, 4, 512).transpose(2, 1, 0, 3)))
    in_maps = []
    for c in range(NCORES):
        b, g = divmod(c, 4)
        f0 = g * NH * HD  # first feature col of this head group (256 wide)
        wq = w_qkv[:, f0:f0 + 256]
        wk = w_qkv[:, CDIM + f0:CDIM + f0 + 256]
        wv = w_qkv[:, 2 * CDIM + f0:2 * CDIM + f0 + 256]

        def blk(a):  # [1024, 256] f32 -> [128 p, 8 ct, 256] f16 contiguous
            return np.ascontiguousarray(
                a.astype(np.float16).reshape(8, 128, 256).transpose(1, 0, 2))

        in_maps.append({
            "xt4": xt4[b],
            "wkq01": blk(np.concatenate([wk[:, :128], wq[:, :128]], axis=1)),
            "wv4": blk(wv),
            "wkq23": blk(np.concatenate([wk[:, 128:], wq[:, 128:]], axis=1)),
            "wout2": np.ascontiguousarray(
                w_out[f0:f0 + 256, :].astype(np.float16)
                .reshape(2, 128, CDIM).transpose(1, 0, 2)),
        })
    return in_maps


def kernel(x, w_qkv, b_qkv, w_out, b_out, _trace=False):
    """Full inputs in, full (B, N, C) output out. b_qkv is all-zeros by the
    problem's input spec (fill: zeros); b_out is added on the host."""
    nc = _get_nc()
    in_maps = make_in_maps(x, w_qkv, w_out)
    res = run_bass_kernel_spmd(nc, in_maps, core_ids=list(range(NCORES)),
                               trace=_trace)
    out = np.zeros((B, NSEQ, CDIM), dtype=np.float32)
    for c in range(NCORES):
        out[c // 4] += res.results[c]["y"].astype(np.float32)
    out += np.asarray(b_out, dtype=np.float32)
    if _trace:
        kernel.last_exec_time_ns = res.exec_time_ns
        kernel.last_results = res
    return out


# revision 22
# speedup vs baseline: 1.0209x; 1.0146x over previous
"""Multi-head self-attention (B=2, N=2048, C=1024, H=16) on 8 TRN2 NeuronCores.

Sharding: data-parallel over batch (2) x tensor-parallel over heads (16/4=4 groups).
Core c handles batch b=c//4 and heads [4*(c%4), 4*(c%4)+4).

v5 schedule. The scalar engine (softmax exp: 128 ACTIVATEs x ~1.1us = 142us) is
the critical resource; PE streaming (~137us) hides under it. DMA descriptor
issue costs ~0.7us per dma_start on the issuing engine, so all inputs load via
EIGHT long-line 2D DMAs (SBUF layouts are chosen so each transfer is contiguous
per partition):
  - x^T as [128p][4 sbk][8 ct][512]: one 1MB DMA per seq-superblock (8KB lines)
    on the sync queue, in consumption order -> compute starts after ~1.5MB.
  - w_qkv as [128p][3 blk][8 ct][256] with blocks (k01|q01, v, k23|q23): three
    0.5MB DMAs on the scalar queue, first-needed first. w_out similarly.
Priority bands steer the Tile list scheduler: band 0 = exp-table preload dummy
+ input DMAs + QKV projection groups; high_priority band = the attention
S->exp->PV->normalize chain so a ready S matmul always beats background work;
out-projection per seq-range is emitted at natural priority and fills PE slack.
Attention runs in a skewed (ic, pair) order so pair-1 prerequisites (k23/q23)
are produced in earlier windows' slack; the final out-projection chunk is the
only true tail: its PSUM->SBUF copies alternate vector/scalar engines (ACT is
idle after the last exp) and its PSUM rotation reuses the freed po banks.

Per-core math identical to v1: S^T pairs row-packed into disjoint 64-row PE
groups, P^T = exp(S/8) on ACT, O_aug^T = [V|1]^T P^T accumulated over key tiles
(ones column yields softmax sums), normalize via reciprocal+partition_broadcast,
Y = O_norm @ W_out. Host sums the 4 head-group partials per batch (+b_out).
"""

import contextlib

import numpy as np

import concourse.bass as bass
import concourse.bacc as bacc
import concourse.tile as tile
from concourse import library_config, mybir
from concourse.bass_utils import run_bass_kernel_spmd

B, NSEQ, CDIM, NHEADS, HD = 2, 2048, 1024, 16, 64
NH = 4          # heads per core
NCORES = 8
F32 = mybir.dt.float32
F16 = mybir.dt.float16  # 16-bit matmul dtype (10-bit mantissa, ample range here)
EXP = mybir.ActivationFunctionType.Exp
SCALE = HD ** -0.5
PRIO_OFF = 1_000_000
FEXP_A = 1024.0 * SCALE / 0.6931471805599453  # log2(e)*2^10*scale
FEXP_B = 15.0 * 1024.0 - 44.25                # f16 exp bias - spline shift


def build_program():
    nc = bacc.Bacc("TRN2", target_bir_lowering=False, debug=False)

    xt4 = nc.dram_tensor("xt4", [4, 128, 8, 512], F16, kind="ExternalInput").ap()
    wkq01 = nc.dram_tensor("wkq01", [128, 8, 256], F16, kind="ExternalInput").ap()
    wv4 = nc.dram_tensor("wv4", [128, 8, 256], F16, kind="ExternalInput").ap()
    wkq23 = nc.dram_tensor("wkq23", [128, 8, 256], F16, kind="ExternalInput").ap()
    wout2 = nc.dram_tensor("wout2", [128, 2, CDIM], F16,
                           kind="ExternalInput").ap()
    y = nc.dram_tensor("y", [NSEQ, CDIM], F16, kind="ExternalOutput").ap()

    with tile.TileContext(nc) as tc:
        emit(nc, tc, xt4, wkq01, wv4, wkq23, wout2, y)

    nc.compile()
    return nc


def emit(nc, tc, xt4, wkq01, wv4, wkq23, wout2, y):
    ctx = contextlib.ExitStack()
    with ctx:
        const = ctx.enter_context(tc.tile_pool(name="const", bufs=1))

        # ---- persistent SBUF tensors (DMA lands here directly, f16) ----
        # wqkv blocks: 0 = [k01|q01], 1 = v (4 heads), 2 = [k23|q23]
        wqkv_sb = const.tile([128, 3, 8, 256], F16)         # [p, blk, ct, 256]
        wout_sb = const.tile([128, 2, CDIM], F16)           # [p, kt, 1024]
        xT_sb = const.tile([128, 4, 8, 512], F16)           # [p, sbk, ct, 512]
        qk_sb = const.tile([128, 4, NSEQ], F16)             # dim1: q01,q23,k01,k23
        v_aug = const.tile([128, 16, NH, HD + 1], F16)      # [p, ntile, head, V|1]
        o_sb = const.tile([128, 2, NSEQ], F16)              # normalized O^T, pairs
        dum = const.tile([1, 8], F32)
        dumw = const.tile([128, 512], F16)                  # PE warm-up operand

        nc.gpsimd.load_library(library_config.attn)
        nc.vector.memset(v_aug[:, :, :, HD:HD + 1], 1.0)
        nc.vector.memset(dum, 1.0)
        nc.vector.memset(dumw, 0.0)

        with tc.tile_pool(name="pP", bufs=4) as pP, \
             tc.tile_pool(name="oup", bufs=2) as oup, \
             tc.tile_pool(name="stat", bufs=2) as stat, \
             tc.tile_pool(name="rbc", bufs=4) as rbc, \
             tc.tile_pool(name="shf", bufs=2) as shf, \
             tc.tile_pool(name="yb", bufs=3) as yb, \
             tc.tile_pool(name="psm", bufs=1, space="PSUM") as psm:

            # exp table preload: a dummy ACTIVATE with no data deps runs first
            # on the scalar engine so the ~2.7us ACT_TABLE_LOAD is off the
            # critical path.
            dume = stat.tile([1, 8], F16, tag="dume", name="dume")
            nc.scalar.activation(dume, dum, EXP)

            # ---- input DMAs: long-line transfers, both HWDGE queues, in
            # consumption order. The first seq-superblock goes per-ct so the
            # first QKV groups chase the transfers instead of waiting on one
            # 1MB descriptor.
            for ct in range(8):
                nc.sync.dma_start(xT_sb[:, 0, ct, :], xt4[0, :, ct, :])
            for sbk in range(1, 4):
                nc.sync.dma_start(xT_sb[:, sbk], xt4[sbk])
            nc.scalar.dma_start(wqkv_sb[:, 0], wkq01)
            nc.scalar.dma_start(wqkv_sb[:, 1], wv4)
            nc.scalar.dma_start(wqkv_sb[:, 2], wkq23)
            nc.scalar.dma_start(wout_sb, wout2)

            # PE warm-up: dummy matmuls with no data deps keep the PE busy
            # during the input DMA window so HAM reaches 2.4GHz before real
            # work, and the first real matmuls issue back-to-back warm.
            wrm = psm.tile([128, 512], F32, tag="vp", bufs=1, name="wrm")
            for r in range(10):
                nc.tensor.matmul(wrm, dumw[:, 0:128], dumw,
                                 start=(r == 0), stop=(r == 9))

            TB = {"qk": 1, "vp": 1, "sb": 2, "o0": 1, "o1": 1}

            def qk_group(blk, half, ft, ic, tag):
                ps = psm.tile([128, 512], F32, tag=tag, bufs=TB[tag], name="psqk")
                for ct in range(8):
                    nc.tensor.matmul(
                        ps,
                        wqkv_sb[:, blk, ct, half * 128:half * 128 + 128],
                        xT_sb[:, ic, ct, :],
                        start=(ct == 0), stop=(ct == 7),
                    )
                nc.vector.tensor_copy(qk_sb[:, ft, ic * 512:(ic + 1) * 512], ps)

            def v_group(nt, tag):
                ps = psm.tile([128, 512], F32, tag=tag, bufs=TB[tag], name="psvp")
                for ct in range(8):
                    nc.tensor.matmul(
                        ps[:, 0:NH * HD],
                        xT_sb[:, nt // 4, ct,
                              (nt % 4) * 128:(nt % 4) * 128 + 128],
                        wqkv_sb[:, 1, ct, :],
                        start=(ct == 0), stop=(ct == 7),
                    )
                for h in range(NH):
                    nc.vector.tensor_copy(
                        v_aug[:, nt, h, 0:HD], ps[:, h * HD:(h + 1) * HD]
                    )

            def y_tile(it, tags, engs):
                """Both 512-wide output chunks for one seq tile, one store."""
                y_sb = yb.tile([128, CDIM], F16, tag="ysb", name="ysbt")
                for fc in range(2):
                    psy = psm.tile([128, 512], F32, tag=tags[fc],
                                   bufs=TB[tags[fc]], name="pyt")
                    for pp in range(2):
                        nc.tensor.matmul(
                            psy,
                            o_sb[:, pp, it * 128:(it + 1) * 128],
                            wout_sb[:, pp, fc * 512:(fc + 1) * 512],
                            start=(pp == 0), stop=(pp == 1),
                        )
                    if engs[fc] == "scalar":
                        nc.scalar.copy(y_sb[:, fc * 512:(fc + 1) * 512], psy)
                    else:
                        nc.vector.tensor_copy(
                            y_sb[:, fc * 512:(fc + 1) * 512], psy)
                nc.sync.dma_start(y[it * 128:(it + 1) * 128, :], y_sb)

            # ---- band 0: QKV projection, emitted in the order the skewed
            # attention windows consume it: k01/q01/V for pair-0 windows
            # first, then q01(ic>=1), then k23/q23 for the delayed pair-1
            # windows. Priorities (= emission order) make background PE work
            # track attention consumption.
            tg = ["qk", "vp"]
            t = 1

            def nxt():
                nonlocal t
                t += 1
                return tg[t % 2]

            qk_group(0, 0, 2, 0, nxt())          # k01(0)
            qk_group(0, 1, 0, 0, nxt())          # q01(0)
            for nt in range(0, 4):
                v_group(nt, nxt())
            for sbk in range(1, 4):
                qk_group(0, 0, 2, sbk, nxt())    # k01(sbk)
                for nt in range(4 * sbk, 4 * sbk + 4):
                    v_group(nt, nxt())

            # Late-needed projection groups + finished seq-ranges' y tiles are
            # sprinkled into earlier windows' jt-chunk boundaries (emission
            # position = scheduler priority): background production tracks
            # attention consumption without head-of-line blocking the
            # in-order PE stream. Window order index -> list of closures.
            sprk = {
                0: [lambda: qk_group(0, 1, 0, 1, nxt()),    # q01(1) -> w1
                    lambda: qk_group(2, 0, 3, 0, nxt()),    # k23(0) -> w2
                    lambda: qk_group(2, 0, 3, 1, nxt()),
                    lambda: qk_group(2, 0, 3, 2, nxt())],
                1: [lambda: qk_group(2, 0, 3, 3, nxt()),
                    lambda: qk_group(2, 1, 1, 0, nxt()),    # q23(0) -> w2
                    lambda: qk_group(0, 1, 0, 2, nxt()),    # q01(2) -> w3
                    lambda: qk_group(2, 1, 1, 1, nxt())],   # q23(1) -> w4
                2: [lambda: qk_group(0, 1, 0, 3, nxt()),    # q01(3) -> w5
                    lambda: qk_group(2, 1, 1, 2, nxt()),    # q23(2) -> w6
                    lambda: qk_group(2, 1, 1, 3, nxt())],   # q23(3) -> w7
            }

            # ---- attention: skewed (ic, pair) order — delays each pair-1
            # pass one window so k23/q23/V production absorbs into slack.
            # y(ic) tiles are sprinkled into the NEXT window's jt loop (their
            # deps are a full window old by then — no head-of-line blocking
            # of the in-order PE stream).
            order = [(0, 0), (1, 0), (0, 1), (2, 0), (1, 1), (3, 0),
                     (2, 1), (3, 1)]
            for widx, (ic, p) in enumerate(order):
                pend = sprk.get(widx, [])
                i0 = ic * 512
                last = widx == len(order) - 1
                po = None
                for jc in range(4):  # jt chunks of 4
                    with tc.high_priority(offset=PRIO_OFF):
                        if po is None:
                            po = [psm.tile([128, 512], F32, tag=f"o{e}",
                                           name=f"po{e}")[0:HD + 1, :]
                                  for e in range(2)]
                        for jt in range(4 * jc, 4 * jc + 4):  # key tile (128)
                            ps = psm.tile([128, 1024], F32, tag="sb", bufs=2,
                                          name="pss")
                            for e in range(2):  # row-group packed pair
                                pb = 64 * e
                                nc.tensor.matmul(
                                    ps[:, e * 512:(e + 1) * 512],
                                    qk_sb[pb:pb + 64, 2 + p,
                                          jt * 128:(jt + 1) * 128],
                                    qk_sb[pb:pb + 64, p, i0:i0 + 512],
                                    start=True, stop=True,
                                    tile_position=(pb, 0),
                                )
                            pt = pP.tile([128, 1024], F16, tag="p")
                            nc.scalar.activation(pt, ps, EXP, scale=SCALE)
                            for e in range(2):
                                nc.tensor.matmul(
                                    po[e],
                                    v_aug[:, jt, 2 * p + e, :],
                                    pt[:, e * 512:(e + 1) * 512],
                                    start=(jt == 0), stop=(jt == 15),
                                )
                    if pend:
                        fn = pend.pop(0)
                        if fn is not None:
                            fn()
                with tc.high_priority(offset=PRIO_OFF):
                    # normalize: copy out of PSUM immediately (frees po
                    # banks), then ONE reciprocal + ONE partition_broadcast
                    # over both heads' concatenated sums rows, then multiply.
                    # e=1 first so its o_sb shift DMA overlaps e=0's mul; the
                    # last window's copies split across scalar+vector (ACT is
                    # idle after the final exp).
                    o_us = [None, None]
                    r0 = stat.tile([1, 1024], F32, tag="r0", name="r0")
                    for e in (1, 0):
                        o_u = oup.tile([HD + 1, 512], F32, tag=f"ou{e}",
                                       name=f"ou{e}")
                        o_us[e] = o_u
                        if last and e == 1:
                            nc.scalar.copy(o_u, po[e])
                        else:
                            nc.vector.tensor_copy(o_u, po[e])
                        nc.sync.dma_start(r0[:, e * 512:(e + 1) * 512],
                                          o_u[HD:HD + 1, :])
                    r1 = stat.tile([1, 1024], F32, tag="r1", name="r1")
                    nc.vector.reciprocal_approx_fast(r1, r0)
                    rb = rbc.tile([64, 1024], F32, tag="rb")
                    nc.gpsimd.partition_broadcast(rb, r1)
                    tmp = shf.tile([64, 512], F16, tag="tmp")
                    nc.vector.tensor_mul(tmp, o_us[1][0:64, :],
                                         rb[:, 512:1024])
                    nc.sync.dma_start(o_sb[64:128, p, i0:i0 + 512], tmp)
                    nc.vector.tensor_mul(o_sb[0:64, p, i0:i0 + 512],
                                         o_us[0][0:64, :], rb[:, 0:512])
                for fn in pend:  # leftovers
                    if fn is not None:
                        fn()
                if p == 1:
                    tiles = []
                    for j, it in enumerate(range(4 * ic, 4 * ic + 4)):
                        engs = ("scalar", "vector") if ic >= 2 else (
                            "vector", "vector")
                        if ic == 3 and j % 2 == 1:
                            # only the very last seq range may rotate through
                            # the po banks — they are busy until the final
                            # normalize for any earlier range
                            tags = ("o0", "o1")
                        else:
                            tags = ("qk", "vp")
                        tiles.append(
                            lambda it=it, tags=tags, engs=engs:
                                y_tile(it, tags, engs))
                    # first two into the next window's LATE chunk slots (the
                    # normalize chain incl. the o_sb shift DMA must be done
                    # for real before the in-order PE stream meets them);
                    # last two a full window later.
                    sprk.setdefault(widx + 1, []).extend(
                        [None, None, tiles[0], tiles[1]])
                    sprk.setdefault(widx + 2, []).extend(tiles[2:])
            # the final seq ranges' out-projection is the only true tail
            for k in (len(order), len(order) + 1):
                for fn in sprk.get(k, []):
                    if fn is not None:
                        fn()



_NC = None


def _get_nc():
    global _NC
    if _NC is None:
        _NC = build_program()
    return _NC


def make_in_maps(x, w_qkv, w_out):
    x = np.asarray(x, dtype=np.float32)
    w_qkv = np.asarray(w_qkv, dtype=np.float32)
    w_out = np.asarray(w_out, dtype=np.float32)
    xt4 = []
    for b in range(B):
        xT = x[b].T.astype(np.float16)  # [C, N] = [(ct p), (sbk s)]
        xt4.append(np.ascontiguousarray(
            xT.reshape(8, 128, 4, 512).transpose(2, 1, 0, 3)))
    in_maps = []
    for c in range(NCORES):
        b, g = divmod(c, 4)
        f0 = g * NH * HD  # first feature col of this head group (256 wide)
        wq = w_qkv[:, f0:f0 + 256]
        wk = w_qkv[:, CDIM + f0:CDIM + f0 + 256]
        wv = w_qkv[:, 2 * CDIM + f0:2 * CDIM + f0 + 256]

        def blk(a):  # [1024, 256] f32 -> [128 p, 8 ct, 256] f16 contiguous
            return np.ascontiguousarray(
                a.astype(np.float16).reshape(8, 128, 256).transpose(1, 0, 2))

        in_maps.append({
            "xt4": xt4[b],
            "wkq01": blk(np.concatenate([wk[:, :128], wq[:, :128]], axis=1)),
            "wv4": blk(wv),
            "wkq23": blk(np.concatenate([wk[:, 128:], wq[:, 128:]], axis=1)),
            "wout2": np.ascontiguousarray(
                w_out[f0:f0 + 256, :].astype(np.float16)
                .reshape(2, 128, CDIM).transpose(1, 0, 2)),
        })
    return in_maps


def kernel(x, w_qkv, b_qkv, w_out, b_out, _trace=False):
    """Full inputs in, full (B, N, C) output out. b_qkv is all-zeros by the
    problem's input spec (fill: zeros); b_out is added on the host."""
    nc = _get_nc()
    in_maps = make_in_maps(x, w_qkv, w_out)
    res = run_bass_kernel_spmd(nc, in_maps, core_ids=list(range(NCORES)),
                               trace=_trace)
    out = np.zeros((B, NSEQ, CDIM), dtype=np.float32)
    for c in range(NCORES):
        out[c // 4] += res.results[c]["y"].astype(np.float32)
    out += np.asarray(b_out, dtype=np.float32)
    if _trace:
        kernel.last_exec_time_ns = res.exec_time_ns
        kernel.last_results = res
    return out
